# revision 47
# baseline (speedup 1.0000x reference)
"""Trainium2 Bass kernel for GQA attention block (nn_Attention_36627481101235).

Reference computation (BS=1, SEQ=2048, DIM=4096, 32 q-heads, 8 kv-heads,
head_dim=128):
    q/k/v projections -> interleaved RoPE on q,k -> repeat_kv -> causal
    softmax attention -> output projection.

Sharding: tensor-parallel by heads over 8 cores. Core c gets q-heads
4c..4c+3 and kv-head c (GQA groups stay intact). Each core computes its
partial out = attn_out_c @ wo_c; the host sums the 8 bf16 partials in
f32.

Precision strategy: the QKV and WO projections run as SPLIT-FP8
DoubleRow matmuls — every operand is decomposed host-side (or
on-chip for attn_outT) into an fp8e4m3 hi/lo pair (a ~= hi + lo,
~8-bit mantissa accuracy) and each product is computed as
hi*hi + hi*lo + lo*hi. DoubleRow perf mode contracts TWO 128-row
k-tiles per instruction at 0.5 PE cycles/row, so the three-term
split runs at 3/4 the PE cost of the f32r/bf16 equivalent while
keeping ~1e-3 relative accuracy. Scores run bf16 (q/k rounded at
RoPE evacuation); the softmax value path (P, v) stays bf16.
Quantization scales (SX=16 for activations, SW=256 for weights) are
undone via the rope tables, the exp's scale input (which also
carries 1/sqrt(head_dim)), and the psum-evacuation copy scales.

Per-core structure, one fused pass per 512-wide s-chunk:
  Phase A: QKV projection, contraction over DIM on the partition axis;
    the six weight streams (wq/wk/wv hi+lo) are packed into ONE dram
    tensor so each k-batch is a single DMA (HWDGE descriptor cost
    dominates small transfers); x^T hi/lo stream per k-batch. The 4 q
    psums live in ps6; k/v reuse the two pacc banks, which are idle
    during the k-loop. RoPE is applied during the psum->sbuf
    evacuation by 4 DVE ops per head (the host pre-permutes wq/wk
    columns so RoPE pairs are contiguous 64-row halves; cross-
    partition DVE reads are legal because one operand is PSUM). The
    critical q0/k evacuations are interleaved so their DVE chains
    pipeline. v is PE-transposed into [s,d] tiles.
  Phase B: transposed-score attention. scoresT[k,q] = kT.T @ qT (bf16);
    softmax without max-subtraction (logits are bounded); exp on ACT
    with scale=1/sqrt(dh) -> P (bf16); PV accumulates in psum over key
    tiles. The denominator uses the STATIONARY-P trick: per 128-col
    q-tile, a width-1 matmul (P-tile stationary, ones moving) costs ~1
    PE cycle instead of 512, accumulating [128q, 4] denominator
    columns; the reciprocal is taken on that parallel layout, column-
    transposed into a [1,512] row, and broadcast with a K=1 matmul.
    The normalization multiply also emits the attn_outT hi/lo fp8 pair
    (ACT copy + DVE sub). Causal handling: key tiles above the
    diagonal are skipped; on diagonal tiles dead columns are sliced
    out; one 128x128 additive tril mask covers the boundary block.
    Producer (scores+exp) and consumer (PV+den) passes are split per
    head with the P-tile pool as the software-pipeline window.
  Phase C: out[s,:] = attn_outT.T @ wo, split-fp8 DoubleRow over
    head-tile pairs, wo hi/lo fully resident (loaded after the weight
    pools are released). One 1MB DMA per 128-row s-tile; the final
    tile drains per 512-col slice to shorten the kernel tail.

Scheduling: B(0) is latency-bound (tiny all-diagonal tiles), so it is
deferred into the tail and woven between B(3) head-slices and C
s-tile batches — C's dense matmuls cover both B's latency chains.
The last projection k-batch is emitted output-major so the q0/k RoPE
chains start under the remaining matmul cover; elsewhere remaining
q-head evacuations are emitted inside the attention head loop.

TimelineSim-predicted per-core time ~356us (PE busy 274us = 77%;
split-fp8 cut the matmul floor from ~375us to ~274us). Remaining
idle: startup DMA fill, per-chunk evacuation boundaries, and the
drain tail. Measured end-to-end relative error vs the fp32 reference
~4.5e-3 (tolerance 2e-2).
"""
import numpy as np

import concourse.mybir as mybir
import concourse.tile as tile
from concourse import bacc

BS, SEQ, DIM = 1, 2048, 4096
NH, DH = 4, 128          # q-heads per core, head dim
DQ = NH * DH             # 512
NCORES = 8
P = 128                  # partitions
SC = 512                 # s-chunk width
NSC = SEQ // SC          # 4
NKT = DIM // P           # 32 contraction tiles for projections
F32R = mybir.dt.float32r
F32 = mybir.dt.float32
BF16 = mybir.dt.bfloat16
F16 = mybir.dt.float16
NEG = -1e9


F8 = mybir.dt.float8e4
DR = mybir.MatmulPerfMode.DoubleRow
SX = 16.0         # fp8 quantization scale for x (and attn_out later)
SW = 256.0        # fp8 quantization scale for weights
INV_SCALE = 1.0 / (SX * SW)
RSQRT_DH = 1.0 / float(np.sqrt(DH))
# packed per-k-row weight layout: wqh | wql | wkh | wkl | wvh | wvl
OQH, OQL = 0, DQ
OKH, OKL = 2 * DQ, 2 * DQ + DH
OVH, OVL = 2 * DQ + 2 * DH, 2 * DQ + 3 * DH
WZW = 2 * DQ + 4 * DH


def build_nc(num_devices=NCORES):
    nc = bacc.Bacc("TRN2", target_bir_lowering=False, debug=False,
                   enable_asserts=False, num_devices=num_devices)
    xTh = nc.dram_tensor("xTh", (DIM, SEQ), F8, kind="ExternalInput").ap()
    xTl = nc.dram_tensor("xTl", (DIM, SEQ), F8, kind="ExternalInput").ap()
    wz = nc.dram_tensor("wz", (DIM, WZW), F8, kind="ExternalInput").ap()
    woh = nc.dram_tensor("woh", (DQ, DIM), F8, kind="ExternalInput").ap()
    wol = nc.dram_tensor("wol", (DQ, DIM), F8, kind="ExternalInput").ap()
    ropeA = nc.dram_tensor("ropeA", (P, SEQ), F32R, kind="ExternalInput").ap()
    ropeB = nc.dram_tensor("ropeB", (P, SEQ), F32R, kind="ExternalInput").ap()
    masks = nc.dram_tensor("masks", (P, P), BF16, kind="ExternalInput").ap()
    ones_col = nc.dram_tensor("ones_col", (1, P), BF16, kind="ExternalInput").ap()
    ones128 = nc.dram_tensor("ones128", (P, 1), BF16, kind="ExternalInput").ap()
    ident = nc.dram_tensor("ident", (P, P), BF16, kind="ExternalInput").ap()
    out = nc.dram_tensor("out", (SEQ, DIM), BF16, kind="ExternalOutput").ap()

    with tile.TileContext(nc) as tc:
        with tc.tile_pool(name="persist", bufs=1) as pp, \
             tc.tile_pool(name="ps6", bufs=6, space="PSUM") as ps6, \
             tc.tile_pool(name="pacc", bufs=1, space="PSUM") as pacc:
            kT_sb = pp.tile([P, SEQ], BF16)             # rotated K^T [d, s]
            v_sb = pp.tile([P, SEQ], BF16)              # v tiles [s%128, st*128+d]
            # attn_outT [d, h*SEQ+s], hi/lo fp8 pair (x SX) for the DR wo mm
            aoTh_sb = pp.tile([P, NH * SEQ], F8, tag="aoTh")
            aoTl_sb = pp.tile([P, NH * SEQ], F8, tag="aoTl")
            ones_col_sb = pp.tile([1, P], BF16)         # value SX (16.0)
            ones128_sb = pp.tile([P, 1], BF16)
            ident_sb = pp.tile([P, P], BF16)

            from contextlib import ExitStack
            with tc.tile_pool(name="tab_p", bufs=1) as tab_p, \
                 tc.tile_pool(name="qTc_p", bufs=4) as qTc_p, \
                 tc.tile_pool(name="tmp_p", bufs=3) as tmp_p, \
                 tc.tile_pool(name="pP_p", bufs=8) as pP_p, \
                 tc.tile_pool(name="rec_p", bufs=2) as rec_p:
                inner = ExitStack()
                wq_p = inner.enter_context(tc.tile_pool(name="wq_p", bufs=1))
                wkv_p = inner.enter_context(tc.tile_pool(name="wkv_p", bufs=1))
                xt_p = inner.enter_context(tc.tile_pool(name="xt_p", bufs=3))
                vt_p = inner.enter_context(tc.tile_pool(name="vt_p", bufs=2))
                # weights, k-tile-major columns: col = k*width + local;
                # hi/lo fp8 pairs for DoubleRow split-fp8 matmuls
                wz_sb = wq_p.tile([P, NKT * WZW], F8, tag="wz")
                ropeA_sb = tab_p.tile([P, SEQ], F32R, tag="ra")
                ropeB_sb = tab_p.tile([P, SEQ], F32R, tag="rb")
                masks_sb = tab_p.tile([P, P], BF16, tag="mk")

                def rope_evac(ps_tile, dst_ap, sc, uid):
                    """dst = RoPE(ps_tile), DVE-direct from psum (cross-
                    partition reads are legal when one operand is PSUM)."""
                    cols = slice(sc * SC, (sc + 1) * SC)
                    swp = tmp_p.tile([P, SC], F32R, tag="ropeswp",
                                     name=f"swp{uid}")
                    nc.vector.tensor_mul(swp[0:64, :], ps_tile[64:128, :],
                                         ropeB_sb[0:64, cols])
                    nc.vector.tensor_mul(swp[64:128, :], ps_tile[0:64, :],
                                         ropeB_sb[64:128, cols])
                    nc.vector.tensor_mul(ps_tile[:], ps_tile[:],
                                         ropeA_sb[:, cols])
                    nc.vector.tensor_add(dst_ap, ps_tile[:], swp[:])

                def rope_evac_hybrid(ps_tile, dst_ap, sc, uid):
                    """RoPE evac with the partition swap on ACT (cross-
                    partition psum->sbuf copies) and only 3 DVE ops — used
                    where DVE is the pacing engine (B(0))."""
                    cols = slice(sc * SC, (sc + 1) * SC)
                    swp = tmp_p.tile([P, SC], F32R, tag="ropeswp",
                                     name=f"hswp{uid}")
                    nc.scalar.copy(swp[0:64, :], ps_tile[64:128, :])
                    nc.scalar.copy(swp[64:128, :], ps_tile[0:64, :])
                    nc.vector.tensor_mul(swp[:], swp[:], ropeB_sb[:, cols])
                    nc.vector.tensor_mul(ps_tile[:], ps_tile[:],
                                         ropeA_sb[:, cols])
                    nc.vector.tensor_add(dst_ap, ps_tile[:], swp[:])

                # 3D views for batched k-tile DMAs: [p, ktile, width]
                xTh3 = xTh.rearrange("(t p) m -> p t m", p=P)
                xTl3 = xTl.rearrange("(t p) m -> p t m", p=P)
                wz3 = wz.rearrange("(t p) m -> p t m", p=P)
                wz_sb3 = wz_sb[:].rearrange("p (t m) -> p t m", m=WZW)
                KB = 4  # k-tiles per DMA batch

                def load_xt4(sc, kb):
                    xth4 = xt_p.tile([P, KB * SC], F8, tag="xth",
                                     name=f"xth{sc}_{kb}")
                    xtl4 = xt_p.tile([P, KB * SC], F8, tag="xtl",
                                     name=f"xtl{sc}_{kb}")
                    ksl = slice(kb * KB, (kb + 1) * KB)
                    scl = slice(sc * SC, (sc + 1) * SC)
                    nc.sync.dma_start(
                        xth4[:].rearrange("p (t m) -> p t m", m=SC),
                        xTh3[:, ksl, scl])
                    nc.sync.dma_start(
                        xtl4[:].rearrange("p (t m) -> p t m", m=SC),
                        xTl3[:, ksl, scl])
                    return xth4, xtl4

                state = {"pending": None}
                qTcs = {}
                psAs = {}

                def finalize(h, sc, ao, dct):
                    """normalize head h's attn_outT by SX/denominator and
                    emit the hi/lo fp8 pair for the DoubleRow wo matmul.
                    dct is the [P, 4] per-qtile denominator-column psum
                    (partition = q%128, free = q-tile)."""
                    rec = rec_p.tile([1, SC], BF16, tag="rec",
                                     name=f"rec{sc}_{h}")
                    with nc.allow_low_precision(reason="softmax denom"):
                        nc.vector.reciprocal(rec[:], dct[:])
                    rb = ps6.tile([P, SC], F32, tag="ps6",
                                  name=f"rb{sc}_{h}")
                    # ones_col is SX, so rb = SX/den (fp8 hi scale baked in)
                    nc.tensor.matmul(rb[:], ones_col_sb[:], rec[:],
                                     start=True, stop=True)
                    rb_sb = tmp_p.tile([P, SC], F32, tag="ropest8",
                                       name=f"rbsb{sc}_{h}")
                    nc.scalar.copy(rb_sb[:], rb[:])
                    cols = slice(h * SEQ + sc * SC, h * SEQ + (sc + 1) * SC)
                    aotmp = tmp_p.tile([P, SC], F32, tag="aotmp",
                                       name=f"aotmp{sc}_{h}")
                    nc.vector.tensor_mul(aotmp[:], ao[:], rb_sb[:])
                    nc.scalar.copy(aoTh_sb[:, cols], aotmp[:])
                    nc.vector.tensor_sub(aoTl_sb[:, cols], aotmp[:],
                                         aoTh_sb[:, cols])

                state["finalize"] = finalize

                def B_heads(sc, heads, qTc, psA=None):
                    nkt = 4 * sc + 4

                    # per-kt live column range: diagonal tiles with
                    # alignment a have columns [0, 128a) fully masked —
                    # skip them in scores/exp/dcol/PV entirely
                    def lo_of(kt):
                        return 128 * (kt - 4 * sc) if kt >= 4 * sc else 0

                    for h in heads:
                        ao = pacc.tile([P, SC], F32, tag="ao",
                                       name=f"ao{sc}_{h}")
                        dct = pacc.tile([1, SC], F32, tag="dcol",
                                        name=f"dcol{sc}_{h}")
                        # producer pass (scores + exp) first, consumers after:
                        # the P-tile pool depth is the software-pipeline window
                        Pts = []
                        for kt in range(nkt):
                            lo = lo_of(kt)
                            S = ps6.tile([P, SC], F32, tag="ps6",
                                         name=f"S{sc}_{h}_{kt}")
                            nc.tensor.matmul(
                                S[:, lo:], kT_sb[:, kt * P:(kt + 1) * P],
                                qTc[:, h * SC + lo:(h + 1) * SC],
                                start=True, stop=True)
                            if kt >= 4 * sc:
                                # triangular mask on the 128-wide diagonal blk
                                nc.vector.tensor_add(
                                    S[:, lo:lo + P], S[:, lo:lo + P],
                                    masks_sb[:])
                            Pt = pP_p.tile([P, SC], BF16, tag="P",
                                           name=f"P{sc}_{h}_{kt}")
                            # 1/sqrt(DH) folded into the exp's scale (the
                            # projections no longer carry it)
                            nc.scalar.activation(
                                Pt[:, lo:], S[:, lo:],
                                mybir.ActivationFunctionType.Exp,
                                scale=RSQRT_DH)
                            Pts.append(Pt)
                            if kt == 1 and state["pending"] is not None:
                                # finalize the previous head here: ps6 still
                                # has free slots (emitting later deadlocks on
                                # the S/P/ao slot cycle)
                                finalize(*state["pending"])
                                state["pending"] = None
                        if psA is not None and h + 1 < NH:
                            # next head's RoPE evac: DVE work that hides
                            # under this head's consumer matmuls; B(0) is
                            # DVE-paced, so there the swap goes to ACT
                            ev = rope_evac_hybrid if sc == 0 else rope_evac
                            ev(psA[h + 1],
                               qTc[:, (h + 1) * SC:(h + 2) * SC], sc,
                               f"{sc}_{h + 1}")
                        if state["pending"] is not None:
                            finalize(*state["pending"])
                            state["pending"] = None
                        for kt in range(nkt):
                            lo = lo_of(kt)
                            nc.tensor.matmul(
                                ao[:, lo:], v_sb[:, kt * P:(kt + 1) * P],
                                Pts[kt][:, lo:],
                                start=(kt == 0), stop=(kt == nkt - 1))
                            nc.tensor.matmul(
                                dct[:, lo:], ones128_sb[:], Pts[kt][:, lo:],
                                start=(kt == 0), stop=(kt == nkt - 1))
                        state["pending"] = (h, sc, ao, dct)
                    if heads[-1] == NH - 1:
                        finalize(*state["pending"])
                        state["pending"] = None

                prefetched = {}
                for sc in range(NSC):
                    scols = slice(sc * SC, (sc + 1) * SC)
                    # ---------- Phase A: QKV projection for this s-chunk ----
                    # q psums from ps6 (4 slots); k/v reuse the pacc banks,
                    # which are idle during the k-loop — leaves 2 ps6 slots
                    # for the previous chunk's attention to drain into
                    psA = [ps6.tile([P, SC], F32, tag="ps6", name=f"psA{sc}_{j}")
                           for j in range(4)]
                    psA.append(pacc.tile([P, SC], F32, tag="ao",
                                         name=f"psA{sc}_4"))
                    psA.append(pacc.tile([P, SC], F32, tag="dcol",
                                         name=f"psA{sc}_5"))
                    for kb in range(NKT // KB):
                        if sc == 0:
                            # stream weights so the first matmuls start as
                            # soon as slice 0 lands
                            ksl = slice(kb * KB, (kb + 1) * KB)
                            nc.sync.dma_start(wz_sb3[:, ksl, :],
                                              wz3[:, ksl, :])
                            xt4 = load_xt4(sc, kb)
                        else:
                            xt4 = prefetched.pop((sc, kb), None)
                            if xt4 is None:
                                xt4 = load_xt4(sc, kb)
                        xth4, xtl4 = xt4
                        xth43 = xth4[:].rearrange("p (t m) -> p t m", m=SC)
                        xtl43 = xtl4[:].rearrange("p (t m) -> p t m", m=SC)

                        def mm_at(j, pi):
                            # split-fp8 DoubleRow: one k-tile PAIR per matmul,
                            # three matmuls (hh, hl, lh) recover bf16-level
                            # precision at 3/4 the billed PE time of f32r
                            p2 = kb * (KB // 2) + pi
                            tsl = slice(2 * pi, 2 * pi + 2)
                            ks2 = slice(2 * p2, 2 * p2 + 2)
                            xh = xth43[:, tsl, :]
                            xl = xtl43[:, tsl, :]
                            if j < NH:
                                oh, ol = OQH + j * DH, OQL + j * DH
                            elif j == 4:
                                oh, ol = OKH, OKL
                            else:
                                oh, ol = OVH, OVL
                            wh = wz_sb3[:, ks2, oh:oh + DH]
                            wl = wz_sb3[:, ks2, ol:ol + DH]
                            first = (p2 == 0)
                            last = (p2 == NKT // 2 - 1)
                            nc.tensor.matmul(psA[j][:], wh, xh,
                                             start=first, stop=False,
                                             perf_mode=DR)
                            nc.tensor.matmul(psA[j][:], wh, xl,
                                             start=False, stop=False,
                                             perf_mode=DR)
                            nc.tensor.matmul(psA[j][:], wl, xh,
                                             start=False, stop=last,
                                             perf_mode=DR)

                        if kb == NKT // KB - 1:
                            # last batch output-major: q0 (then k) finish
                            # accumulating first, so their RoPE evacuation
                            # chains start under the remaining matmul cover
                            for j in (0, 4, 5, 1, 2, 3):
                                for pi in range(KB // 2):
                                    mm_at(j, pi)
                        else:
                            for pi in range(KB // 2):
                                for j in range(6):
                                    mm_at(j, pi)
                        if sc == 0 and kb == 4:
                            # rope/mask tables and consts are first needed at
                            # the evac / in B(0) — keep them off the startup
                            # critical path
                            nc.sync.dma_start(ropeA_sb[:], ropeA[:])
                            nc.sync.dma_start(ropeB_sb[:], ropeB[:])
                            nc.sync.dma_start(masks_sb[:], masks[:])
                            nc.sync.dma_start(ones_col_sb[:], ones_col[:])
                            nc.sync.dma_start(ones128_sb[:], ones128[:])
                            nc.sync.dma_start(ident_sb[:], ident[:])
                    qTc = qTc_p.tile([P, NH * SC], BF16, tag="qTc")
                    # prefetch the next chunk's first x tiles: the DMA queue
                    # is idle during the evacuations and B
                    if sc + 1 < NSC:
                        for pkb in range(2):
                            prefetched[(sc + 1, pkb)] = load_xt4(sc + 1, pkb)
                    # evacuate q-head 0 and k first (they gate B's first
                    # scores): hybrid ACT swaps + interleaved DVE chains so
                    # the two critical evacuations pipeline across engines
                    swq = tmp_p.tile([P, SC], F32R, tag="ropeswp",
                                     name=f"dsq{sc}")
                    swk = tmp_p.tile([P, SC], F32R, tag="ropeswp",
                                     name=f"dsk{sc}")
                    nc.vector.tensor_mul(swq[0:64, :], psA[0][64:128, :],
                                         ropeB_sb[0:64, scols])
                    nc.vector.tensor_mul(swq[64:128, :], psA[0][0:64, :],
                                         ropeB_sb[64:128, scols])
                    nc.vector.tensor_mul(psA[0][:], psA[0][:],
                                         ropeA_sb[:, scols])
                    nc.vector.tensor_add(qTc[:, 0:SC], psA[0][:], swq[:])
                    nc.vector.tensor_mul(swk[0:64, :], psA[4][64:128, :],
                                         ropeB_sb[0:64, scols])
                    nc.vector.tensor_mul(swk[64:128, :], psA[4][0:64, :],
                                         ropeB_sb[64:128, scols])
                    nc.vector.tensor_mul(psA[4][:], psA[4][:],
                                         ropeA_sb[:, scols])
                    nc.vector.tensor_add(kT_sb[:, scols], psA[4][:], swk[:])
                    vtmp = vt_p.tile([P, SC], BF16, tag="vtmp")
                    # v psum carries the fp8 quantization scale; undo it here
                    nc.scalar.activation(vtmp[:], psA[5][:],
                                         mybir.ActivationFunctionType.Copy,
                                         scale=INV_SCALE)
                    for t in range(4):
                        ptr = ps6.tile([P, P], BF16, tag="ps6",
                                       name=f"ptr{sc}_{t}")
                        nc.tensor.transpose(ptr[:], vtmp[:, t * P:(t + 1) * P],
                                            ident_sb[:])
                        nc.scalar.copy(
                            v_sb[:, (sc * 4 + t) * P:(sc * 4 + t + 1) * P],
                            ptr[:])
                    # ---------- Phase B: attention -------------------------
                    # B(0) is latency-bound (tiny all-diagonal tiles), so it
                    # is deferred into the tail where C's dense matmuls hide
                    # its ACT/DVE chains; its q1-3 evacuate eagerly here
                    # (DVE is idle during chunk 1's k-loop)
                    qTcs[sc] = qTc
                    psAs[sc] = psA
                    if sc == 0:
                        for j in range(1, NH):
                            rope_evac(psA[j], qTc[:, j * SC:(j + 1) * SC],
                                      sc, f"{sc}_{j}")
                    elif sc < NSC - 1:
                        B_heads(sc, (0, 1, 2, 3), qTc, psA)

                # free the projection weights/x pools before phase C so wo
                # can be resident while B(3) runs
                inner.close()

                # ------ Phase B(3) woven with phase C ----------------------
                # C s-tiles 0..11 depend only on B(0..2); interleave them
                # with B(3)'s heads to fill its latency chains
                with tc.tile_pool(name="wo_p", bufs=1) as wo_p, \
                     tc.tile_pool(name="out_p", bufs=3) as out_p:
                    woh_t = wo_p.tile([P, 4 * DIM], F8, tag="woh")
                    wol_t = wo_p.tile([P, 4 * DIM], F8, tag="wol")
                    nc.sync.dma_start(
                        woh_t[:].rearrange("p (t m) -> p t m", m=DIM),
                        woh.rearrange("(t p) m -> p t m", p=P))
                    nc.sync.dma_start(
                        wol_t[:].rearrange("p (t m) -> p t m", m=DIM),
                        wol.rearrange("(t p) m -> p t m", p=P))
                    aoTh3 = aoTh_sb[:].rearrange("p (t m) -> p t m", m=SEQ)
                    aoTl3 = aoTl_sb[:].rearrange("p (t m) -> p t m", m=SEQ)
                    woh3 = woh_t[:].rearrange("p (t m) -> p t m", m=DIM)
                    wol3 = wol_t[:].rearrange("p (t m) -> p t m", m=DIM)

                    def C_st(sts):
                        for st in sts:
                            ot = out_p.tile([P, DIM], BF16, tag="ot",
                                            name=f"ot{st}")
                            last = st == SEQ // P - 1
                            ssl = slice(st * P, (st + 1) * P)
                            for dc in range(8):
                                po = ps6.tile([P, SC], F32, tag="ps6",
                                              name=f"po{st}_{dc}")
                                dsl = slice(dc * SC, (dc + 1) * SC)
                                for hp in range(2):
                                    hsl = slice(2 * hp, 2 * hp + 2)
                                    ah = aoTh3[:, hsl, ssl]
                                    al = aoTl3[:, hsl, ssl]
                                    wh = woh3[:, hsl, dsl]
                                    wl = wol3[:, hsl, dsl]
                                    nc.tensor.matmul(po[:], ah, wh,
                                                     start=(hp == 0),
                                                     stop=False, perf_mode=DR)
                                    nc.tensor.matmul(po[:], ah, wl,
                                                     start=False, stop=False,
                                                     perf_mode=DR)
                                    nc.tensor.matmul(po[:], al, wh,
                                                     start=False,
                                                     stop=(hp == 1),
                                                     perf_mode=DR)
                                nc.scalar.activation(
                                    ot[:, dsl], po[:],
                                    mybir.ActivationFunctionType.Copy,
                                    scale=INV_SCALE)
                                if last:
                                    # drain the final s-tile per dc-slice so
                                    # the kernel tail isn't one long DMA
                                    nc.sync.dma_start(
                                        out[st * P:(st + 1) * P, dsl],
                                        ot[:, dsl])
                            if not last:
                                nc.sync.dma_start(
                                    out[st * P:(st + 1) * P, :], ot[:])

                    # weave B(3), the deferred B(0), and C: the C s-tiles
                    # only need their own chunk's aoT, so each B slice
                    # unlocks the next C batch while C's matmuls cover the
                    # B latency chains
                    B_heads(NSC - 1, (0,), qTcs[NSC - 1], psAs[NSC - 1])
                    B_heads(0, (0, 1, 2, 3), qTcs[0], None)
                    C_st([0, 1, 2, 3])
                    B_heads(NSC - 1, (1,), qTcs[NSC - 1], psAs[NSC - 1])
                    C_st([4, 5, 6, 7])
                    B_heads(NSC - 1, (2,), qTcs[NSC - 1], psAs[NSC - 1])
                    C_st([8, 9, 10, 11])
                    B_heads(NSC - 1, (3,), qTcs[NSC - 1], psAs[NSC - 1])
                    C_st([12, 13, 14, 15])
    nc.compile()
    return nc


def make_in_maps(x, freqs_cos, freqs_sin, wq, wk, wv, wo):
    """Host-side sharding + layout prep. Returns list of 8 per-core dicts."""
    import ml_dtypes
    bf16 = np.dtype(ml_dtypes.bfloat16)
    f8 = np.dtype(ml_dtypes.float8_e4m3)
    f32 = np.float32

    def split8(a, s):
        """hi/lo fp8 pair: a*s ~= hi + lo with ~8-bit mantissa accuracy."""
        hi = (a * s).astype(f8)
        lo = ((a * s) - hi.astype(f32)).astype(f8)
        return hi, lo

    x2 = np.asarray(x, f32).reshape(SEQ, DIM)
    xT = np.ascontiguousarray(x2.T)
    xTh, xTl = split8(xT, SX)
    # RoPE de-interleave permutation within each head: evens then odds
    perm = np.concatenate([np.arange(0, DH, 2), np.arange(1, DH, 2)])
    cosT = np.ascontiguousarray(np.asarray(freqs_cos, f32).T)   # [64, SEQ]
    sinT = np.ascontiguousarray(np.asarray(freqs_sin, f32).T)
    # fp8 quantization scale (SX*SW) undone via the rope tables
    ropeA = np.concatenate([cosT, cosT], axis=0) * INV_SCALE    # [128, SEQ]
    ropeB = np.concatenate([-sinT, sinT], axis=0) * INV_SCALE
    # 4 causal mask alignment patterns: a-th block [128, 512]:
    # keep (0) where qq - 128a - kk >= 0 else -1e9
    kk = np.arange(P)[:, None]
    qq = np.arange(P)[None, :]
    masks = np.where(qq - kk >= 0, 0.0, NEG).astype(bf16)
    ones_col = np.full((1, P), SX, bf16)  # bakes the aoT fp8 hi scale into rb
    ones128 = np.ones((P, 1), bf16)
    ident = np.eye(P, dtype=bf16)

    wq_f = np.asarray(wq, f32)
    wk_f = np.asarray(wk, f32)
    wv_f = np.asarray(wv, f32)
    wo_f = np.asarray(wo, f32)
    in_maps = []
    for c in range(NCORES):
        wq_c = wq_f[:, c * DQ:(c + 1) * DQ].reshape(DIM, NH, DH)[:, :, perm]
        wq_c = np.ascontiguousarray(wq_c.reshape(DIM, DQ))
        wk_c = np.ascontiguousarray(wk_f[:, c * DH:(c + 1) * DH][:, perm])
        wv_c = np.ascontiguousarray(wv_f[:, c * DH:(c + 1) * DH])
        wo_c = np.ascontiguousarray(wo_f[c * DQ:(c + 1) * DQ, :])
        wqh_c, wql_c = split8(wq_c, SW)
        wkh_c, wkl_c = split8(wk_c, SW)
        wvh_c, wvl_c = split8(wv_c, SW)
        woh_c, wol_c = split8(wo_c, SW)
        wz_c = np.ascontiguousarray(np.concatenate(
            [wqh_c, wql_c, wkh_c, wkl_c, wvh_c, wvl_c], axis=1))
        in_maps.append({
            "xTh": xTh, "xTl": xTl,
            "wz": wz_c, "woh": woh_c, "wol": wol_c,
            "ropeA": ropeA, "ropeB": ropeB, "masks": masks,
            "ones_col": ones_col, "ones128": ones128, "ident": ident,
        })
    return in_maps


_NC_CACHE = None


def kernel(x, freqs_cos, freqs_sin, mask, wq, wk, wv, wo):
    """Full-input entry point: returns [1, 2048, 4096] float32."""
    global _NC_CACHE
    from concourse.bass_utils import run_bass_kernel_spmd
    if _NC_CACHE is None:
        _NC_CACHE = build_nc()
    in_maps = make_in_maps(x, freqs_cos, freqs_sin, wq, wk, wv, wo)
    res = run_bass_kernel_spmd(_NC_CACHE, in_maps, core_ids=list(range(NCORES)))
    acc = np.zeros((SEQ, DIM), np.float32)
    for c in range(NCORES):
        acc += res.results[c]["out"].astype(np.float32)
    return acc.reshape(BS, SEQ, DIM)

